# revision 1
# baseline (speedup 1.0000x reference)
"""AntiSymmetricConv (GNN message passing) on 8 TRN2 NeuronCores.

Strategy (dst-sharded "pull" mode):
  - Host: compute deg/dinv, sort dst nodes by degree (desc), assign 128-node
    tiles round-robin to 8 cores (load balance + identical static schedule),
    build per-(tile, slot, k) source-index arrays: slot p of a tile holds the
    k-th in-edge of that tile's p-th dst.  Because slot p <-> dst p, the
    scatter-add becomes PSUM accumulation with a *static identity* stationary
    operand - no per-chunk one-hot matrices.
  - Device, per iteration:
      phase A: per node tile: xT = transpose(x_tile) (PE), [xw|xa] = x_tile @
               [W_phi.T | A.T] (one matmul, N=256), y = xw * dinv * valid,
               xa += bias; write y to DRAM bounce.
      AllGather y shards -> y_full [8*NPC, 128] (Shared DRAM).
      phase C: per dst tile: indirect-DMA gather in-edge rows of y_full into
               SBUF [128, D*128], matmul-accumulate (lhsT = identity) in
               groups of 4 chunks (N=512) into one PSUM bank, + self-loop
               matmul from local y; epilogue folds PSUM blocks, h =
               tanh(xa + dinv*agg), x += 0.1*h.
  - Output: read back x shards, invert the permutation on host.
"""

import os

import numpy as np

import concourse.bacc as bacc
import concourse.bass as bass
import concourse.mybir as mybir
import concourse.tile as tile
from concourse.bass_utils import run_bass_kernel_spmd
from concourse.masks import make_identity

GAMMA = 0.1
EPSILON = 0.1
NUM_ITERS = 4
P = 128  # partitions / tile size
C = 8    # cores
D = 128  # feature dim

FP = mybir.dt.float32
I32 = mybir.dt.int32


# ----------------------------------------------------------------- host prep
def _preprocess(x, edge_index, W, W_phi, bias):
    N = x.shape[0]
    E = edge_index.shape[1]
    src, dst = edge_index[0].astype(np.int64), edge_index[1].astype(np.int64)

    deg = np.bincount(dst, minlength=N).astype(np.float64) + 1.0
    dinv = (1.0 / np.sqrt(deg)).astype(np.float32)

    # global degree-descending order of dst nodes
    order = np.argsort(-deg, kind="stable")
    rank = np.empty(N, dtype=np.int64)
    rank[order] = np.arange(N)

    n_tiles_global = -(-N // P)          # 782 for N=100000
    # +1 guarantees at least one all-pad slot (the ZERO row) on core C-1
    NT = -(-(n_tiles_global + 1) // C)   # tiles per core (98)
    NPC = NT * P                         # rows per core (12544)

    # node (by rank r) -> (core, tile_i, slot)
    g = rank // P
    core_of = g % C
    tile_of = g // C
    slot_of = rank % P

    # position of each node's y-row inside y_full ([core][slot][tile] layout,
    # row-major [128, NT, 128] per core => row index = slot*NT + tile)
    pos = core_of * NPC + slot_of * NT + tile_of  # int64 [N]
    ZERO_POS = np.int32((C - 1) * NPC + NPC - 1)  # last row of core 7: all-pad

    # per-edge target placement
    er = rank[dst]
    # order edges by (core, tile, slot) then assign k within each dst
    eorder = np.argsort(er, kind="stable")
    er_s = er[eorder]
    src_s = src[eorder]
    # k-th edge of each dst: running counter within equal er_s
    # (er_s sorted -> use index - first-occurrence)
    first = np.searchsorted(er_s, er_s)
    k_of = np.arange(E) - first

    # per-(core, tile) max in-edge count, then schedule = max over cores
    indeg = (deg - 1.0).astype(np.int64)
    indeg_sorted = indeg[order]  # by rank
    pad_tiles = NT * C - n_tiles_global
    indeg_pad = np.concatenate(
        [indeg_sorted, np.zeros(NT * C * P - N, dtype=np.int64)]
    )
    tile_max = indeg_pad.reshape(NT * C, P).max(axis=1)  # per global tile g
    D_sched = tile_max.reshape(NT, C).max(axis=1)        # per tile position i
    D_sched = np.maximum(D_sched, 1).astype(np.int64)
    CHT = int(D_sched.sum())

    # chunk-column offsets per tile position
    off = np.zeros(NT + 1, dtype=np.int64)
    off[1:] = np.cumsum(D_sched)

    # build src index arrays [C][P, CHT]
    src_arr = np.full((C, P, CHT), ZERO_POS, dtype=np.int32)
    eg = er_s // P                       # global tile of each (sorted) edge
    e_tile = eg // C
    e_core_s = eg % C
    e_slot = er_s % P
    col = off[e_tile] + k_of
    src_arr[e_core_s, e_slot, col] = pos[src_s].astype(np.int32)

    # per-core node data: x_sb[c][p, t*D+f] = x[node at (c, slot p, tile t)]
    node_ids = np.full((C, P, NT), -1, dtype=np.int64)
    node_ids[core_of, slot_of, tile_of] = np.arange(N)
    valid = node_ids >= 0
    nid = np.where(valid, node_ids, 0)
    x_gather = x[nid.reshape(C, -1)]  # [C, P*NT, D]
    x_gather[~valid.reshape(C, -1)] = 0.0
    x_sb = x_gather.reshape(C, P, NT, D).reshape(C, P, NT * D)
    dv = dinv[nid]
    dinv_sb = np.where(valid, dv, 1.0).astype(np.float32)
    dinv_y_sb = np.where(valid, dv, 0.0).astype(np.float32)

    # matmul RHS [128, 256] = [W_phi.T | A.T], A = W - W.T - GAMMA*I
    A = W - W.T - GAMMA * np.eye(D, dtype=np.float32)
    rhs = np.concatenate([W_phi.T, A.T], axis=1).astype(np.float32)
    bias_bcast = np.tile(bias[None, :], (P, 1)).astype(np.float32)

    in_maps = []
    for c in range(C):
        in_maps.append(
            {
                "x_in": np.ascontiguousarray(x_sb[c]),
                "dinv": np.ascontiguousarray(dinv_sb[c]),
                "dinv_y": np.ascontiguousarray(dinv_y_sb[c]),
                "src_idx": np.ascontiguousarray(src_arr[c]),
                "rhs": rhs,
                "bias_b": bias_bcast,
            }
        )
    meta = dict(
        NT=NT, NPC=NPC, D_sched=[int(v) for v in D_sched], CHT=CHT,
        node_ids=node_ids, valid=valid, N=N,
    )
    return in_maps, meta


def _postprocess(results, meta):
    NT, N = meta["NT"], meta["N"]
    node_ids, valid = meta["node_ids"], meta["valid"]
    out = np.empty((N, D), dtype=np.float32)
    for c in range(C):
        xc = results[c]["x_out"].reshape(P, NT, D)
        v = valid[c]
        out[node_ids[c][v]] = xc[v]
    return out


# ------------------------------------------------------------- device graph
def _build_graph(NT, D_sched, n_iters=NUM_ITERS, skip_collective=False):
    NPC = NT * P
    CHT = int(sum(D_sched))
    GMAX = 12  # max chunks gathered per indirect DMA (SBUF budget)

    nc = bacc.Bacc("TRN2", target_bir_lowering=False, debug=False, num_devices=C)
    x_in = nc.declare_dram_parameter("x_in", [P, NT * D], FP, isOutput=False)
    dinv_in = nc.declare_dram_parameter("dinv", [P, NT], FP, isOutput=False)
    dinv_y_in = nc.declare_dram_parameter("dinv_y", [P, NT], FP, isOutput=False)
    src_in = nc.declare_dram_parameter("src_idx", [P, CHT], I32, isOutput=False)
    rhs_in = nc.declare_dram_parameter("rhs", [P, 2 * D], FP, isOutput=False)
    bias_in = nc.declare_dram_parameter("bias_b", [P, D], FP, isOutput=False)
    x_out = nc.declare_dram_parameter("x_out", [P, NT * D], FP, isOutput=True)

    y_bounce = nc.dram_tensor("y_bounce", [NPC, D], FP)
    y_full = nc.dram_tensor("y_full", [C * NPC, D], FP, addr_space="Shared")

    off = np.zeros(NT + 1, dtype=np.int64)
    off[1:] = np.cumsum(D_sched)

    with tile.TileContext(nc) as tc:
        with (
            tc.tile_pool(name="stat", bufs=1) as stat,
            tc.tile_pool(name="sb", bufs=2) as sb,
            tc.tile_pool(name="gat", bufs=4) as gat,
            tc.tile_pool(name="ps", bufs=2, space="PSUM") as psp,
            tc.tile_pool(name="psagg", bufs=2, space="PSUM") as psagg,
        ):
            # ---- static data
            ident = stat.tile([P, P], FP)
            make_identity(nc, ident[:])
            rhs_sb = stat.tile([P, 2 * D], FP)
            nc.sync.dma_start(rhs_sb[:], rhs_in[:])
            bias_sb = stat.tile([P, D], FP)
            nc.sync.dma_start(bias_sb[:], bias_in[:])
            dinv_sb = stat.tile([P, NT], FP)
            nc.sync.dma_start(dinv_sb[:], dinv_in[:])
            dinvy_sb = stat.tile([P, NT], FP)
            nc.sync.dma_start(dinvy_sb[:], dinv_y_in[:])
            idx_sb = stat.tile([P, CHT], I32)
            nc.sync.dma_start(idx_sb[:], src_in[:])
            x_sb = stat.tile([P, NT * D], FP)
            nc.sync.dma_start(x_sb[:], x_in[:])
            y_sb = stat.tile([P, NT * D], FP)
            xa_sb = stat.tile([P, NT * D], FP)

            def phase_a(_iv):
                # ---------------- phase A: local matmuls
                for i in range(NT):
                    xt = x_sb[:, i * D:(i + 1) * D]
                    ps_t = psp.tile([P, P], FP, tag="ps_t", space="PSUM")
                    nc.tensor.transpose(out=ps_t[:], in_=xt, identity=ident[:])
                    xT = sb.tile([P, P], FP, tag="xT")
                    nc.vector.tensor_copy(out=xT[:], in_=ps_t[:])
                    ps_a = psp.tile([P, 2 * D], FP, tag="ps_a", space="PSUM")
                    nc.tensor.matmul(
                        out=ps_a[:], lhsT=xT[:], rhs=rhs_sb[:],
                        start=True, stop=True,
                    )
                    # y = xw * dinv * valid
                    nc.vector.tensor_scalar(
                        out=y_sb[:, i * D:(i + 1) * D], in0=ps_a[:, 0:D],
                        scalar1=dinvy_sb[:, i:i + 1], scalar2=None,
                        op0=mybir.AluOpType.mult,
                    )
                    # xa = x@A.T + bias
                    nc.vector.tensor_tensor(
                        out=xa_sb[:, i * D:(i + 1) * D], in0=ps_a[:, D:2 * D],
                        in1=bias_sb[:], op=mybir.AluOpType.add,
                    )
                # y rows: y_bounce[slot*NT + tile] = y_sb[slot, tile*D:...]
                # flat copy: y_sb [P, NT*D] -> y_bounce [NPC, D] row-major is
                # exactly the same bytes laid out [P][NT][D] -> row p*NT+t. ✓
                nc.sync.dma_start(
                    out=y_bounce[:].rearrange("(p t) d -> p (t d)", p=P),
                    in_=y_sb[:],
                )
                if skip_collective:
                    # timing-only variant: local copy into own shard slot
                    nc.sync.dma_start(
                        out=y_full[0:NPC, :], in_=y_bounce[:],
                    )
                else:
                    nc.gpsimd.collective_compute(
                        "AllGather",
                        mybir.AluOpType.bypass,
                        replica_groups=[list(range(C))],
                        ins=[y_bounce[:].opt()],
                        outs=[y_full[:].opt()],
                    )

            def phase_c(_iv):
                # ---------------- phase C: gather + aggregate per dst tile
                for i in range(NT):
                    Di = int(D_sched[i])
                    ps_g = psagg.tile([P, D], FP, tag="agg", space="PSUM")
                    # one [P,1]->[P,D] indirect gather per chunk (proven
                    # HW semantics), matmul-accumulate into PSUM
                    for k in range(Di):
                        yb = gat.tile([P, D], FP, tag="ybig")
                        nc.gpsimd.indirect_dma_start(
                            out=yb[:],
                            out_offset=None,
                            in_=y_full[:],
                            in_offset=bass.IndirectOffsetOnAxis(
                                ap=idx_sb[:, off[i] + k: off[i] + k + 1],
                                axis=0,
                            ),
                        )
                        nc.tensor.matmul(
                            out=ps_g[:], lhsT=ident[:], rhs=yb[:],
                            start=(k == 0), stop=False,
                        )
                    # self-loop: + y_tile
                    nc.tensor.matmul(
                        out=ps_g[:], lhsT=ident[:],
                        rhs=y_sb[:, i * D:(i + 1) * D],
                        start=False, stop=True,
                    )
                    # epilogue: t3 = agg * dinv ; t4 = t3 + xa ; h = tanh(t4)
                    t3 = sb.tile([P, D], FP, tag="t3")
                    nc.vector.tensor_scalar(
                        out=t3[:], in0=ps_g[:], scalar1=dinv_sb[:, i:i + 1],
                        scalar2=None, op0=mybir.AluOpType.mult,
                    )
                    t4 = sb.tile([P, D], FP, tag="t4")
                    nc.vector.tensor_tensor(
                        out=t4[:], in0=t3[:], in1=xa_sb[:, i * D:(i + 1) * D],
                        op=mybir.AluOpType.add,
                    )
                    h = sb.tile([P, D], FP, tag="h")
                    nc.scalar.activation(
                        out=h[:], in_=t4[:],
                        func=mybir.ActivationFunctionType.Tanh,
                    )
                    h1 = sb.tile([P, D], FP, tag="h1")
                    nc.scalar.activation(
                        out=h1[:], in_=h[:],
                        func=mybir.ActivationFunctionType.Copy, scale=EPSILON,
                    )
                    nc.vector.tensor_tensor(
                        out=x_sb[:, i * D:(i + 1) * D],
                        in0=x_sb[:, i * D:(i + 1) * D], in1=h1[:],
                        op=mybir.AluOpType.add,
                    )

            for _it in range(n_iters):
                phase_a(_it)
                # 1-trip loop: the back-edge resets the SWDGE sem lane,
                # which otherwise overflows its 16-bit wait field after
                # ~4095 indirect DMAs in straight-line code.
                with tc.For_i(0, 1, 1) as _iv:
                    phase_c(_iv)
            nc.sync.dma_start(out=x_out[:], in_=x_sb[:])
    nc.compile()
    return nc


# ------------------------------------------------------------------- driver
_LAST = {}


def kernel(x, edge_index, W, W_phi, bias):
    x = np.asarray(x, dtype=np.float32)
    edge_index = np.asarray(edge_index, dtype=np.int32)
    W = np.asarray(W, dtype=np.float32)
    W_phi = np.asarray(W_phi, dtype=np.float32)
    bias = np.asarray(bias, dtype=np.float32)

    in_maps, meta = _preprocess(x, edge_index, W, W_phi, bias)
    nc = _build_graph(meta["NT"], meta["D_sched"])
    trace = os.environ.get("BASS_PROFILE", "0") == "1"
    res = run_bass_kernel_spmd(
        nc, in_maps, core_ids=list(range(C)), trace=trace
    )
    _LAST["res"] = res
    _LAST["meta"] = meta
    return _postprocess(res.results, meta)

